# revision 6
# baseline (speedup 1.0000x reference)
"""Trainium2 8-core GQA attention kernel (tensor-parallel over heads).

Strategy (8 NeuronCores, SPMD):
  - Core c owns q-heads [4c..4c+4) and kv-head c (GQA groups stay aligned).
  - Phase A: qkvT = wqkv_c^T @ x^T computed feature-major so Q^T/K^T land in
    [head_dim, tokens] layout; RoPE applied with partition-shifted multiply-adds.
  - Phase B: scores computed transposed (S^T[k,q]) so exp(S^T) feeds the PV
    matmul directly (lhsT = V[k,d]) with zero P transposes; softmax denominator
    via a ones-column matmul; normalization deferred to the output.
  - Phase C: AllGather the per-core attention outputs (feature-major, bf16),
    then each core computes a 512-column slice of out = attn @ wo.
  - Host: shards/casts inputs, transposes x, concatenates output slices.
All PE math in bf16 (f32 PSUM accumulation); masks/softmax pieces in f32.
"""

import numpy as np
import ml_dtypes

import concourse.bass as bass
import concourse.mybir as mybir
import concourse.tile as tile
from concourse import bacc
from concourse.bass_utils import run_bass_kernel_spmd

BF16 = mybir.dt.bfloat16
F32 = mybir.dt.float32
HD = 128            # head dim
HHD = HD // 2       # rope half
P = 128             # partitions
QCH = 512           # q-chunk (phase B free dim)
NCH = 512           # token chunk (phase A free dim)
TCH = 512           # token chunk (phase C free dim)
KT = 128            # k tile (partition dim)
NEG = -1.0e30
SCALE = 1.0 / np.sqrt(HD)


def build_graph(NB, S, D, HPC, NCORES, block_cls, n_mixed):
    """Build the per-core SPMD graph.

    NB: batches, S: seq len, D: model dim, HPC: q heads per core.
    block_cls[(qc, kt)] -> 'full' | 'skip' | int (mixed-mask slot index)
    """
    TOK = NB * S
    QF = HPC * HD           # q features per core
    FLOC = QF + 2 * HD      # local qkv features (q + k + v)
    MT = FLOC // P          # feature tiles (q tiles + 1 k + 1 v)
    KD = D // P             # contraction tiles over model dim
    NQC = S // QCH          # q chunks per batch
    NKT = S // KT           # k tiles per batch
    ODPC = D // NCORES      # output dims per core
    n_mask = max(n_mixed, 1)

    nc = bacc.Bacc("TRN2", target_bir_lowering=False, debug=False,
                   num_devices=NCORES)

    xt_d = nc.dram_tensor("xt", [D, TOK], BF16, kind="ExternalInput").ap()
    wqkv_d = nc.dram_tensor("wqkv", [D, FLOC], BF16, kind="ExternalInput").ap()
    wo_d = nc.dram_tensor("wo", [D, ODPC], BF16, kind="ExternalInput").ap()
    sc_d = nc.dram_tensor("sincos2", [P, 2 * S], BF16, kind="ExternalInput").ap()
    mask_d = nc.dram_tensor("maskblk", [n_mask * P, QCH], BF16,
                            kind="ExternalInput").ap()
    out_d = nc.dram_tensor("out", [ODPC, TOK], F32, kind="ExternalOutput").ap()

    with tile.TileContext(nc) as tc:
        with tc.tile_pool(name="persist", bufs=1) as persist, \
             tc.tile_pool(name="dram", bufs=1, space="DRAM") as dram:
            qkvT = persist.tile([P, MT, TOK], BF16)
            v_kd = persist.tile([P, NB * NKT, HD], BF16)
            ident = persist.tile([P, P], BF16)
            ones_col = persist.tile([P, 1], BF16)
            ones_row = persist.tile([1, P], BF16)
            nc.vector.memset(ones_col[:], 1.0)
            nc.vector.memset(ones_row[:], 1.0)
            nc.gpsimd.memset(ident[:], 0.0)
            nc.gpsimd.affine_select(
                out=ident[:], in_=ident[:],
                compare_op=mybir.AluOpType.not_equal, fill=1.0, base=0,
                pattern=[[-1, P]], channel_multiplier=1)

            attn_bounce = dram.tile([QF, TOK], BF16)
            ag_out = dram.tile([QF * NCORES, TOK], BF16,
                               addr_space="Shared" if NCORES > 4 else "Local")

            # ---------------- Phase A: qkvT = wqkv^T @ x^T, + RoPE ----------
            with tc.tile_pool(name="pha", bufs=1) as pha, \
                 tc.tile_pool(name="phax", bufs=2) as phax, \
                 tc.tile_pool(name="phat", bufs=2) as phat, \
                 tc.tile_pool(name="psa", bufs=1, space="PSUM") as psa:
                wq_sb = pha.tile([P, KD, FLOC], BF16)
                nc.sync.dma_start(
                    wq_sb[:], wqkv_d.rearrange("(ko p) m -> p ko m", p=P))
                sc_sb = pha.tile([P, 2 * S], BF16)
                nc.sync.dma_start(sc_sb[:], sc_d[:])
                cosT = sc_sb[:, 0:S]
                sinT = sc_sb[:, S:2 * S]

                KH = KD // 2  # stream x^T in two half-contraction chunks
                for ch in range(TOK // NCH):
                    col0 = ch * NCH
                    pss = [psa.tile([P, NCH], F32, tag="pa", bufs=MT + 1,
                                    name=f"pa{ch}_{m}")
                           for m in range(MT)]
                    for half in range(2):
                        xt_sb = phax.tile([P, KH, NCH], BF16, tag="xt")
                        nc.sync.dma_start(
                            xt_sb[:],
                            xt_d[half * KH * P:(half + 1) * KH * P,
                                 col0:col0 + NCH]
                            .rearrange("(ko p) t -> p ko t", p=P))
                        for k in range(KH):
                            kg = half * KH + k
                            for m in range(MT):
                                nc.tensor.matmul(
                                    pss[m][:],
                                    wq_sb[:, kg, m * P:(m + 1) * P],
                                    xt_sb[:, k, :],
                                    start=(kg == 0), stop=(kg == KD - 1))
                    # RoPE for q tiles and the k tile; plain copy for v
                    s0 = col0 % S
                    for m in range(MT):
                        dst = qkvT[:, m, col0:col0 + NCH]
                        if m == MT - 1:  # v
                            nc.vector.tensor_copy(dst, pss[m][:])
                            continue
                        t1 = phat.tile([P, NCH], F32, tag="t1")
                        t2 = phat.tile([P, NCH], F32, tag="t2")
                        nc.vector.tensor_mul(t1[:], pss[m][:],
                                             cosT[:, s0:s0 + NCH])
                        nc.vector.tensor_mul(t2[0:HHD, :],
                                             pss[m][HHD:P, :],
                                             sinT[0:HHD, s0:s0 + NCH])
                        nc.vector.tensor_mul(t2[HHD:P, :],
                                             pss[m][0:HHD, :],
                                             sinT[HHD:P, s0:s0 + NCH])
                        nc.vector.tensor_add(dst, t1[:], t2[:])

                # V^T -> V via PE transposes (bf16)
                for b in range(NB):
                    for kt in range(NKT):
                        pt_ps = psa.tile([P, P], BF16, tag="vt", bufs=1)
                        nc.tensor.transpose(
                            pt_ps[:],
                            qkvT[:, MT - 1,
                                 b * S + kt * KT:b * S + (kt + 1) * KT],
                            ident[:])
                        nc.vector.tensor_copy(v_kd[:, b * NKT + kt, :],
                                              pt_ps[:])

            # ---------------- Phase B: attention ---------------------------
            with tc.tile_pool(name="phb", bufs=1) as phb, \
                 tc.tile_pool(name="phbw", bufs=3) as phbw, \
                 tc.tile_pool(name="psb", bufs=1, space="PSUM") as psb:
                if n_mixed > 0:
                    mk_sb = phb.tile([P, n_mask, QCH], BF16)
                    nc.sync.dma_start(
                        mk_sb[:], mask_d.rearrange("(mb p) q -> p mb q", p=P))
                for b in range(NB):
                    for qc in range(NQC):
                        kts = [kt for kt in range(NKT)
                               if block_cls[(qc, kt)] != 'skip']
                        q0 = b * S + qc * QCH
                        for h in range(HPC):
                            o_ps = psb.tile([P, QCH], F32, tag="outT", bufs=2)
                            d_ps = psb.tile([1, QCH], F32, tag="den", bufs=2)
                            for i, kt in enumerate(kts):
                                st = psb.tile([P, QCH], F32, tag="st", bufs=2)
                                nc.tensor.matmul(
                                    st[:],
                                    qkvT[:, HPC,
                                         b * S + kt * KT:b * S + (kt + 1) * KT],
                                    qkvT[:, h, q0:q0 + QCH],
                                    start=True, stop=True)
                                cls = block_cls[(qc, kt)]
                                if cls != 'full':
                                    nc.vector.tensor_add(
                                        st[:], st[:], mk_sb[:, cls, :])
                                pt = phbw.tile([P, QCH], BF16, tag="pt")
                                nc.scalar.activation(
                                    pt[:], st[:],
                                    mybir.ActivationFunctionType.Exp,
                                    bias=0.0, scale=float(SCALE))
                                first, last = (i == 0), (i == len(kts) - 1)
                                nc.tensor.matmul(
                                    o_ps[:], v_kd[:, b * NKT + kt, :], pt[:],
                                    start=first, stop=last)
                                nc.tensor.matmul(
                                    d_ps[:], ones_col[:], pt[:],
                                    start=first, stop=last)
                            inv = phbw.tile([1, QCH], F32, tag="inv")
                            nc.vector.reciprocal(inv[:], d_ps[:])
                            invb = phbw.tile([1, QCH], BF16, tag="invb")
                            nc.vector.tensor_copy(invb[:], inv[:])
                            bc_ps = psb.tile([P, QCH], F32, tag="bc", bufs=1)
                            nc.tensor.matmul(bc_ps[:], ones_row[:], invb[:],
                                             start=True, stop=True)
                            bcc = phbw.tile([P, QCH], BF16, tag="bcc")
                            nc.vector.tensor_copy(bcc[:], bc_ps[:])
                            at = phbw.tile([P, QCH], BF16, tag="at")
                            nc.vector.tensor_mul(at[:], o_ps[:], bcc[:])
                            nc.sync.dma_start(
                                attn_bounce[h * P:(h + 1) * P, q0:q0 + QCH],
                                at[:])

            # ---------------- AllGather ------------------------------------
            nc.gpsimd.collective_compute(
                "AllGather", mybir.AluOpType.bypass,
                replica_groups=[list(range(NCORES))],
                ins=[attn_bounce.opt()], outs=[ag_out.opt()])

            # ---------------- Phase C: out_cT = wo_c^T @ attn^T -------------
            with tc.tile_pool(name="phc", bufs=1) as phc, \
                 tc.tile_pool(name="phcx", bufs=2) as phcx, \
                 tc.tile_pool(name="phco", bufs=2) as phco, \
                 tc.tile_pool(name="psc", bufs=1, space="PSUM") as psc:
                wo_sb = phc.tile([P, KD, ODPC], BF16)
                nc.sync.dma_start(
                    wo_sb[:], wo_d.rearrange("(ko p) m -> p ko m", p=P))
                for tcx in range(TOK // TCH):
                    c0 = tcx * TCH
                    agt = phcx.tile([P, KD, TCH], BF16, tag="agt")
                    nc.sync.dma_start(
                        agt[:],
                        ag_out[:, c0:c0 + TCH]
                        .rearrange("(ko p) t -> p ko t", p=P))
                    for md in range(ODPC // P):
                        po = psc.tile([P, TCH], F32, tag="po", bufs=3)
                        for kf in range(KD):
                            nc.tensor.matmul(
                                po[:],
                                wo_sb[:, kf, md * P:(md + 1) * P],
                                agt[:, kf, :],
                                start=(kf == 0), stop=(kf == KD - 1))
                        osb = phco.tile([P, TCH], F32, tag="osb")
                        nc.vector.tensor_copy(osb[:], po[:])
                        nc.sync.dma_start(
                            out_d[md * P:(md + 1) * P, c0:c0 + TCH], osb[:])

    nc.compile()
    return nc


def _host_prep(x, wqkv, wo, sincos, full_causal_mask, start_pos,
               NB, S, D, HPC, NCORES):
    """Shard, cast, and lay out inputs; classify mask blocks."""
    bf16 = ml_dtypes.bfloat16
    TOK = NB * S
    H = HPC * NCORES
    QF = HPC * HD
    NQC = S // QCH
    NKT = S // KT
    ODPC = D // NCORES
    q_sz = H * HD

    xt = np.ascontiguousarray(x.reshape(TOK, D).T).astype(bf16)

    # effective mask: [q, k] (batch-shared), incl. the cache-validity term
    m_eff = np.asarray(full_causal_mask[0, 0], dtype=bool)
    m_eff = m_eff[start_pos:start_pos + S, :S].copy()
    valid = np.arange(S) < (start_pos + S)
    m_eff &= valid[None, :]

    block_cls = {}
    mixed_blocks = []
    for qc in range(NQC):
        for kt in range(NKT):
            blk = m_eff[qc * QCH:(qc + 1) * QCH, kt * KT:(kt + 1) * KT]
            if blk.all():
                block_cls[(qc, kt)] = 'full'
            elif not blk.any():
                block_cls[(qc, kt)] = 'skip'
            else:
                block_cls[(qc, kt)] = len(mixed_blocks)
                add = np.where(blk, 0.0, NEG).astype(np.float32)
                mixed_blocks.append(np.ascontiguousarray(add.T))  # [k, q]
    n_mixed = len(mixed_blocks)
    if n_mixed:
        maskblk = np.concatenate(mixed_blocks, axis=0).astype(bf16)
    else:
        maskblk = np.zeros((P, QCH), dtype=bf16)

    # rope tables, transposed + duplicated halves; sin rows 0:64 negated
    sc = np.asarray(sincos[start_pos:start_pos + S], dtype=np.float32)
    sin, cos = sc[:, :HHD], sc[:, HHD:]
    cosT2 = np.concatenate([cos.T, cos.T], axis=0)           # [128, S]
    sinT2 = np.concatenate([-sin.T, sin.T], axis=0)          # [128, S]
    sincos2 = np.concatenate([cosT2, sinT2], axis=1).astype(bf16)

    in_maps = []
    for c in range(NCORES):
        qcols = np.asarray(wqkv[:, c * QF:(c + 1) * QF])
        kcols = np.asarray(wqkv[:, q_sz + c * HD:q_sz + (c + 1) * HD])
        vcols = np.asarray(
            wqkv[:, q_sz + NCORES * HD + c * HD:
                 q_sz + NCORES * HD + (c + 1) * HD])
        wqkv_c = np.concatenate([qcols, kcols, vcols], axis=1).astype(bf16)
        wo_c = np.ascontiguousarray(
            np.asarray(wo[:, c * ODPC:(c + 1) * ODPC])).astype(bf16)
        in_maps.append({
            "xt": xt, "wqkv": wqkv_c, "wo": wo_c,
            "sincos2": sincos2, "maskblk": maskblk,
        })
    return in_maps, block_cls, n_mixed


_CACHE = {}


def run_distributed(x, wqkv, wo, sincos, full_causal_mask, start_pos,
                    NB, S, D, HPC, NCORES, trace=False, tmpdir=None):
    in_maps, block_cls, n_mixed = _host_prep(
        x, wqkv, wo, sincos, full_causal_mask, start_pos,
        NB, S, D, HPC, NCORES)
    key = (NB, S, D, HPC, NCORES,
           tuple(sorted((k, v) for k, v in block_cls.items())))
    if key not in _CACHE:
        _CACHE[key] = build_graph(NB, S, D, HPC, NCORES, block_cls, n_mixed)
    nc = _CACHE[key]
    res = run_bass_kernel_spmd(nc, in_maps, list(range(NCORES)), trace=trace,
                               tmpdir=tmpdir)
    TOK = NB * S
    out = np.empty((TOK, D), dtype=np.float32)
    ODPC = D // NCORES
    for c in range(NCORES):
        out[:, c * ODPC:(c + 1) * ODPC] = res.results[c]["out"].T
    return out.reshape(NB, S, D), res


def kernel(x, wqkv, wo, sincos, cache_k, cache_v, full_causal_mask,
           start_pos) -> np.ndarray:
    x = np.asarray(x)
    start_pos = int(np.asarray(start_pos))
    B, S_, D_ = x.shape
    assert start_pos == 0, "prefill-only kernel (seq fills the whole cache)"
    out, _ = run_distributed(
        x, np.asarray(wqkv), np.asarray(wo), np.asarray(sincos),
        np.asarray(full_causal_mask), start_pos,
        NB=B, S=S_, D=D_, HPC=4, NCORES=8)
    return out


# revision 12
# speedup vs baseline: 1.1533x; 1.1533x over previous
"""Trainium2 8-core GQA attention kernel (tensor-parallel over heads).

Strategy (8 NeuronCores, SPMD):
  - Core c owns q-heads [4c..4c+4) and kv-head c (GQA groups stay aligned).
  - Phase A: qkvT = wqkv_c^T @ x^T computed feature-major so Q^T/K^T land in
    [head_dim, tokens] layout; RoPE applied with partition-shifted multiply-adds.
  - Phase B: scores computed transposed (S^T[k,q]) so exp(S^T) feeds the PV
    matmul directly (lhsT = V[k,d]) with zero P transposes; causal blocks that
    are fully masked are skipped, partially-masked blocks get a multiplicative
    {0,1} bf16 mask after the exp; softmax denominators for all 4 heads
    accumulate into one [4, 512] PSUM row-set via indicator-column matmuls;
    normalization is deferred to the output (broadcast via tiny matmuls).
  - The AllGather of attention outputs is split into 8 token-chunk collectives
    software-pipelined with phase B (producer) and phase C (consumer).
  - Phase C: each core computes a 512-row slice of out^T = wo_c^T @ attn^T.
  - Host: shards/casts inputs, transposes x, concatenates output slices.
All PE math in bf16 (f32 PSUM accumulation).
"""

import numpy as np
import ml_dtypes

import concourse.bass as bass
import concourse.mybir as mybir
import concourse.tile as tile
from concourse import bacc
from concourse.bass_utils import run_bass_kernel_spmd

BF16 = mybir.dt.bfloat16
F32 = mybir.dt.float32
HD = 128            # head dim
HHD = HD // 2       # rope half
P = 128             # partitions
QCH = 512           # q-chunk / token-chunk size
KT = 128            # k tile (partition dim)
SCALE = 1.0 / np.sqrt(HD)


def build_graph(NB, S, D, HPC, NCORES, block_cls, n_mixed):
    """Build the per-core SPMD graph.

    block_cls[(qc, kt)] -> 'full' | 'skip' | int (mixed-mask slot index)
    """
    TOK = NB * S
    QF = HPC * HD           # q features per core
    FLOC = QF + 2 * HD      # local qkv features (q + k + v)
    MT = FLOC // P          # feature tiles (q tiles + 1 k + 1 v)
    KD = D // P             # contraction tiles over model dim
    NQC = S // QCH          # q chunks per batch
    NKT = S // KT           # k tiles per batch
    ODPC = D // NCORES      # output dims per core
    NCHK = TOK // QCH       # token chunks overall
    n_mask = max(n_mixed, 1)

    nc = bacc.Bacc("TRN2", target_bir_lowering=False, debug=False,
                   num_devices=NCORES)

    xt_d = nc.dram_tensor("xt", [D, TOK], BF16, kind="ExternalInput").ap()
    wqkv_d = nc.dram_tensor("wqkv", [D, FLOC], BF16, kind="ExternalInput").ap()
    wo_d = nc.dram_tensor("wo", [D, ODPC], BF16, kind="ExternalInput").ap()
    sc_d = nc.dram_tensor("sincos2", [P, 2 * S], BF16, kind="ExternalInput").ap()
    mask_d = nc.dram_tensor("maskblk", [n_mask * P, QCH], BF16,
                            kind="ExternalInput").ap()
    eye_d = nc.dram_tensor("eye", [HPC, HPC * P], BF16,
                           kind="ExternalInput").ap()
    out_d = nc.dram_tensor("out", [ODPC, TOK], F32, kind="ExternalOutput").ap()

    with tile.TileContext(nc) as tc:
        with tc.tile_pool(name="persist", bufs=1) as persist, \
             tc.tile_pool(name="dram", bufs=1, space="DRAM") as dram:
            qkvT = persist.tile([P, MT, TOK], BF16)
            v_kd = persist.tile([P, NB * NKT, HD], BF16)
            ident = persist.tile([P, P], BF16)
            nc.gpsimd.memset(ident[:], 0.0)
            nc.gpsimd.affine_select(
                out=ident[:], in_=ident[:],
                compare_op=mybir.AluOpType.not_equal, fill=1.0, base=0,
                pattern=[[-1, P]], channel_multiplier=1)
            # indicator columns/rows for per-head denominator batching
            ecol = persist.tile([P, HPC, HPC], BF16)   # [:, h, :] = e_h cols
            erow = persist.tile([HPC, HPC, P], BF16)   # [:, h, :] = e_h rows
            nc.vector.memset(ecol[:], 0.0)
            for h in range(HPC):
                nc.vector.memset(ecol[:, h, h:h + 1], 1.0)
            nc.sync.dma_start(erow[:], eye_d[:])

            bounce = [dram.tile([QF, QCH], BF16, name=f"bnc{ci}")
                      for ci in range(NCHK)]
            agc = [dram.tile([QF * NCORES, QCH], BF16, name=f"agc{ci}",
                             addr_space="Shared" if NCORES > 4 else "Local")
                   for ci in range(NCHK)]

            # ---------------- Phase A: qkvT = wqkv^T @ x^T, + RoPE ----------
            with tc.tile_pool(name="pha", bufs=1) as pha, \
                 tc.tile_pool(name="phax", bufs=2) as phax, \
                 tc.tile_pool(name="phat", bufs=2) as phat, \
                 tc.tile_pool(name="psa", bufs=1, space="PSUM") as psa:
                wq_sb = pha.tile([P, KD, FLOC], BF16)
                for ko in range(KD):
                    nc.sync.dma_start(
                        wq_sb[:, ko, :],
                        wqkv_d[ko * P:(ko + 1) * P, :])
                sc_sb = pha.tile([P, 2 * S], BF16)
                nc.sync.dma_start(sc_sb[:], sc_d[:])
                cosT = sc_sb[:, 0:S]
                sinT = sc_sb[:, S:2 * S]

                KH = KD // 2  # stream x^T in two half-contraction chunks
                for b in range(NB):
                    for chb in range(S // QCH):
                        ch = b * (S // QCH) + chb
                        col0 = ch * QCH
                        pss = [psa.tile([P, QCH], F32, tag="pa", bufs=MT + 1,
                                         name=f"pa{ch}_{m}")
                               for m in range(MT)]
                        for half in range(2):
                            xt_sb = phax.tile([P, KH, QCH], BF16, tag="xt")
                            nc.sync.dma_start(
                                xt_sb[:],
                                xt_d[half * KH * P:(half + 1) * KH * P,
                                     col0:col0 + QCH]
                                .rearrange("(ko p) t -> p ko t", p=P))
                            for k in range(KH):
                                kg = half * KH + k
                                for m in range(MT):
                                    nc.tensor.matmul(
                                        pss[m][:],
                                        wq_sb[:, kg, m * P:(m + 1) * P],
                                        xt_sb[:, k, :],
                                        start=(kg == 0), stop=(kg == KD - 1))
                        # RoPE for q tiles and the k tile; plain copy for v
                        s0 = col0 % S
                        for m in range(MT):
                            dst = qkvT[:, m, col0:col0 + QCH]
                            if m == MT - 1:  # v
                                nc.vector.tensor_copy(dst, pss[m][:])
                                continue
                            t1 = phat.tile([P, QCH], F32, tag="t1")
                            t2 = phat.tile([P, QCH], F32, tag="t2")
                            nc.vector.tensor_mul(t1[:], pss[m][:],
                                                 cosT[:, s0:s0 + QCH])
                            nc.vector.tensor_mul(t2[0:HHD, :],
                                                 pss[m][HHD:P, :],
                                                 sinT[0:HHD, s0:s0 + QCH])
                            nc.vector.tensor_mul(t2[HHD:P, :],
                                                 pss[m][0:HHD, :],
                                                 sinT[HHD:P, s0:s0 + QCH])
                            nc.vector.tensor_add(dst, t1[:], t2[:])
                    # V^T -> V via PE transposes (bf16), per batch
                    for kt in range(NKT):
                        pt_ps = psa.tile([P, P], BF16, tag="vt", bufs=1,
                                          name=f"vt{b}_{kt}")
                        nc.tensor.transpose(
                            pt_ps[:],
                            qkvT[:, MT - 1,
                                 b * S + kt * KT:b * S + (kt + 1) * KT],
                            ident[:])
                        nc.vector.tensor_copy(v_kd[:, b * NKT + kt, :],
                                              pt_ps[:])

            # ------------- Phases B + AG + C, software-pipelined ------------
            with tc.tile_pool(name="phb", bufs=1) as phb, \
                 tc.tile_pool(name="phbw", bufs=3) as phbw, \
                 tc.tile_pool(name="phcx", bufs=2) as phcx, \
                 tc.tile_pool(name="phco", bufs=2) as phco, \
                 tc.tile_pool(name="psbc", bufs=1, space="PSUM") as psbc:
                wo_sb = phb.tile([P, KD, ODPC], BF16)
                for ko in range(KD):
                    nc.sync.dma_start(
                        wo_sb[:, ko, :], wo_d[ko * P:(ko + 1) * P, :])
                if n_mixed > 0:
                    mk_sb = phb.tile([P, n_mask, QCH], BF16)
                    nc.sync.dma_start(
                        mk_sb[:], mask_d.rearrange("(mb p) q -> p mb q", p=P))

                def phase_c_chunk(ci):
                    agt = phcx.tile([P, KD, QCH], BF16, tag="agt",
                                    name=f"agt{ci}")
                    nc.sync.dma_start(
                        agt[:],
                        agc[ci].rearrange("(ko p) t -> p ko t", p=P))
                    for md in range(ODPC // P):
                        po = psbc.tile([P, QCH], F32, tag="po", bufs=2,
                                       name=f"po{ci}_{md}")
                        for kf in range(KD):
                            nc.tensor.matmul(
                                po[:],
                                wo_sb[:, kf, md * P:(md + 1) * P],
                                agt[:, kf, :],
                                start=(kf == 0), stop=(kf == KD - 1))
                        osb = phco.tile([P, QCH], F32, tag="osb")
                        nc.vector.tensor_copy(osb[:], po[:])
                        nc.sync.dma_start(
                            out_d[md * P:(md + 1) * P,
                                  ci * QCH:(ci + 1) * QCH], osb[:])

                # software pipeline: B(ci) -> AG(ci) while C(ci-1) runs
                _o_tiles = {}

                def phase_b_chunk2(ci):
                    b, qc = divmod(ci, NQC)
                    kts = [kt for kt in range(NKT)
                           if block_cls[(qc, kt)] != 'skip']
                    q0 = b * S + qc * QCH
                    d_ps = psbc.tile([HPC, QCH], F32, tag="den", bufs=1,
                                     name=f"den{ci}")
                    for h in range(HPC):
                        o_ps = psbc.tile([P, QCH], F32, tag="outT", bufs=2,
                                         name=f"o{ci}_{h}")
                        for i, kt in enumerate(kts):
                            st = psbc.tile([P, QCH], F32, tag="st", bufs=2,
                                           name=f"st{ci}_{h}_{i}")
                            nc.tensor.matmul(
                                st[:],
                                qkvT[:, HPC,
                                     b * S + kt * KT:b * S + (kt + 1) * KT],
                                qkvT[:, h, q0:q0 + QCH],
                                start=True, stop=True)
                            pt = phbw.tile([P, QCH], BF16, tag="pt", bufs=6,
                                           name=f"pt{ci}_{h}_{i}")
                            nc.scalar.activation(
                                pt[:], st[:],
                                mybir.ActivationFunctionType.Exp,
                                bias=0.0, scale=float(SCALE))
                            cls = block_cls[(qc, kt)]
                            if cls != 'full':
                                nc.vector.tensor_mul(pt[:], pt[:],
                                                     mk_sb[:, cls, :])
                            first, last = (i == 0), (i == len(kts) - 1)
                            nc.tensor.matmul(
                                o_ps[:], v_kd[:, b * NKT + kt, :], pt[:],
                                start=first, stop=last)
                            nc.tensor.matmul(
                                d_ps[:], ecol[:, h, :], pt[:],
                                start=(first and h == 0),
                                stop=(last and h == HPC - 1))
                        o_sb = phbw.tile([P, QCH], BF16, tag="osbuf", bufs=5,
                                         name=f"ou{ci}_{h}")
                        nc.vector.tensor_copy(o_sb[:], o_ps[:])
                        _o_tiles[(ci, h)] = o_sb
                    inv = phbw.tile([HPC, QCH], F32, tag="inv",
                                    name=f"inv{ci}")
                    nc.vector.reciprocal(inv[:], d_ps[:])
                    invb = phbw.tile([HPC, QCH], BF16, tag="invb",
                                     name=f"invb{ci}")
                    nc.vector.tensor_copy(invb[:], inv[:])
                    # normalize + stage each head's block to the bounce buffer
                    for h in range(HPC):
                        bc_ps = psbc.tile([P, QCH], F32, tag="bc", bufs=1,
                                          name=f"bc{ci}_{h}")
                        nc.tensor.matmul(bc_ps[:], erow[:, h, :], invb[:],
                                         start=True, stop=True)
                        bcc = phbw.tile([P, QCH], BF16, tag="bcc",
                                        name=f"bcc{ci}_{h}")
                        nc.vector.tensor_copy(bcc[:], bc_ps[:])
                        at = phbw.tile([P, QCH], BF16, tag="at",
                                       name=f"at{ci}_{h}")
                        nc.vector.tensor_mul(at[:], _o_tiles[(ci, h)][:],
                                             bcc[:])
                        nc.sync.dma_start(
                            bounce[ci][h * P:(h + 1) * P, :], at[:])

                for ci in range(NCHK):
                    phase_b_chunk2(ci)
                    nc.gpsimd.collective_compute(
                        "AllGather", mybir.AluOpType.bypass,
                        replica_groups=[list(range(NCORES))],
                        ins=[bounce[ci].opt()], outs=[agc[ci].opt()])
                    if ci >= 1:
                        phase_c_chunk(ci - 1)
                phase_c_chunk(NCHK - 1)

    nc.compile()
    return nc


def _host_prep(x, wqkv, wo, sincos, full_causal_mask, start_pos,
               NB, S, D, HPC, NCORES):
    """Shard, cast, and lay out inputs; classify mask blocks."""
    bf16 = ml_dtypes.bfloat16
    TOK = NB * S
    H = HPC * NCORES
    QF = HPC * HD
    NQC = S // QCH
    NKT = S // KT
    ODPC = D // NCORES
    q_sz = H * HD

    xt = np.ascontiguousarray(x.reshape(TOK, D).T).astype(bf16)

    # effective mask: [q, k] (batch-shared), incl. the cache-validity term
    m_eff = np.asarray(full_causal_mask[0, 0], dtype=bool)
    m_eff = m_eff[start_pos:start_pos + S, :S].copy()
    valid = np.arange(S) < (start_pos + S)
    m_eff &= valid[None, :]

    block_cls = {}
    mixed_blocks = []
    for qc in range(NQC):
        for kt in range(NKT):
            blk = m_eff[qc * QCH:(qc + 1) * QCH, kt * KT:(kt + 1) * KT]
            if blk.all():
                block_cls[(qc, kt)] = 'full'
            elif not blk.any():
                block_cls[(qc, kt)] = 'skip'
            else:
                block_cls[(qc, kt)] = len(mixed_blocks)
                mixed_blocks.append(
                    np.ascontiguousarray(blk.T.astype(np.float32)))  # [k, q]
    n_mixed = len(mixed_blocks)
    if n_mixed:
        maskblk = np.concatenate(mixed_blocks, axis=0).astype(bf16)
    else:
        maskblk = np.zeros((P, QCH), dtype=bf16)

    # rope tables, transposed + duplicated halves; sin rows 0:64 negated
    sc = np.asarray(sincos[start_pos:start_pos + S], dtype=np.float32)
    sin, cos = sc[:, :HHD], sc[:, HHD:]
    cosT2 = np.concatenate([cos.T, cos.T], axis=0)           # [128, S]
    sinT2 = np.concatenate([-sin.T, sin.T], axis=0)          # [128, S]
    sincos2 = np.concatenate([cosT2, sinT2], axis=1).astype(bf16)

    eye = np.zeros((HPC, HPC, P), dtype=bf16)
    for h in range(HPC):
        eye[h, h, :] = 1
    eye = eye.reshape(HPC, HPC * P)

    in_maps = []
    for c in range(NCORES):
        qcols = np.asarray(wqkv[:, c * QF:(c + 1) * QF])
        kcols = np.asarray(wqkv[:, q_sz + c * HD:q_sz + (c + 1) * HD])
        vcols = np.asarray(
            wqkv[:, q_sz + NCORES * HD + c * HD:
                 q_sz + NCORES * HD + (c + 1) * HD])
        wqkv_c = np.concatenate([qcols, kcols, vcols], axis=1).astype(bf16)
        wo_c = np.ascontiguousarray(
            np.asarray(wo[:, c * ODPC:(c + 1) * ODPC])).astype(bf16)
        in_maps.append({
            "xt": xt, "wqkv": wqkv_c, "wo": wo_c,
            "sincos2": sincos2, "maskblk": maskblk, "eye": eye,
        })
    return in_maps, block_cls, n_mixed


_CACHE = {}


def run_distributed(x, wqkv, wo, sincos, full_causal_mask, start_pos,
                    NB, S, D, HPC, NCORES, trace=False, tmpdir=None):
    in_maps, block_cls, n_mixed = _host_prep(
        x, wqkv, wo, sincos, full_causal_mask, start_pos,
        NB, S, D, HPC, NCORES)
    key = (NB, S, D, HPC, NCORES,
           tuple(sorted((k, v) for k, v in block_cls.items())))
    if key not in _CACHE:
        _CACHE[key] = build_graph(NB, S, D, HPC, NCORES, block_cls, n_mixed)
    nc = _CACHE[key]
    res = run_bass_kernel_spmd(nc, in_maps, list(range(NCORES)), trace=trace,
                               tmpdir=tmpdir)
    TOK = NB * S
    out = np.empty((TOK, D), dtype=np.float32)
    ODPC = D // NCORES
    for c in range(NCORES):
        out[:, c * ODPC:(c + 1) * ODPC] = res.results[c]["out"].T
    return out.reshape(NB, S, D), res


def kernel(x, wqkv, wo, sincos, cache_k, cache_v, full_causal_mask,
           start_pos) -> np.ndarray:
    x = np.asarray(x)
    start_pos = int(np.asarray(start_pos))
    B, S_, D_ = x.shape
    assert start_pos == 0, "prefill-only kernel (seq fills the whole cache)"
    out, _ = run_distributed(
        x, np.asarray(wqkv), np.asarray(wo), np.asarray(sincos),
        np.asarray(full_causal_mask), start_pos,
        NB=B, S=S_, D=D_, HPC=4, NCORES=8)
    return out


# revision 13
# speedup vs baseline: 1.2082x; 1.0476x over previous
"""Trainium2 8-core GQA attention kernel (tensor-parallel over heads).

Strategy (8 NeuronCores, SPMD):
  - Core c owns q-heads [4c..4c+4) and kv-head c (GQA groups stay aligned).
  - Phase A: qkvT = wqkv_c^T @ x^T computed feature-major so Q^T/K^T land in
    [head_dim, tokens] layout; RoPE applied with partition-shifted multiply-adds.
  - Phase B: scores computed transposed (S^T[k,q]) so exp(S^T) feeds the PV
    matmul directly (lhsT = V[k,d]) with zero P transposes; causal blocks that
    are fully masked are skipped, partially-masked blocks get a multiplicative
    {0,1} bf16 mask after the exp; softmax denominators for all 4 heads
    accumulate into one [4, 512] PSUM row-set via indicator-column matmuls;
    normalization is deferred to the output (broadcast via tiny matmuls).
  - The AllGather of attention outputs is split into 8 token-chunk collectives
    software-pipelined with phase B (producer) and phase C (consumer).
  - Phase C: each core computes a 512-row slice of out^T = wo_c^T @ attn^T.
  - Host: shards/casts inputs, transposes x, concatenates output slices.
All PE math in bf16 (f32 PSUM accumulation).
"""

import numpy as np
import ml_dtypes

import concourse.bass as bass
import concourse.mybir as mybir
import concourse.tile as tile
from concourse import bacc
from concourse.bass_utils import run_bass_kernel_spmd

BF16 = mybir.dt.bfloat16
F32 = mybir.dt.float32
HD = 128            # head dim
HHD = HD // 2       # rope half
P = 128             # partitions
QCH = 512           # q-chunk / token-chunk size
KT = 128            # k tile (partition dim)
SCALE = 1.0 / np.sqrt(HD)


def build_graph(NB, S, D, HPC, NCORES, block_cls, n_mixed):
    """Build the per-core SPMD graph.

    block_cls[(qc, kt)] -> 'full' | 'skip' | int (mixed-mask slot index)
    """
    TOK = NB * S
    QF = HPC * HD           # q features per core
    FLOC = QF + 2 * HD      # local qkv features (q + k + v)
    MT = FLOC // P          # feature tiles (q tiles + 1 k + 1 v)
    KD = D // P             # contraction tiles over model dim
    NQC = S // QCH          # q chunks per batch
    NKT = S // KT           # k tiles per batch
    ODPC = D // NCORES      # output dims per core
    NCHK = TOK // QCH       # token chunks overall
    n_mask = max(n_mixed, 1)

    nc = bacc.Bacc("TRN2", target_bir_lowering=False, debug=False,
                   num_devices=NCORES)

    xt_d = nc.dram_tensor("xt", [D, TOK], BF16, kind="ExternalInput").ap()
    wqkv_d = nc.dram_tensor("wqkv", [D, FLOC], BF16, kind="ExternalInput").ap()
    wo_d = nc.dram_tensor("wo", [D, ODPC], BF16, kind="ExternalInput").ap()
    sc_d = nc.dram_tensor("sincos2", [P, 2 * S], BF16, kind="ExternalInput").ap()
    mask_d = nc.dram_tensor("maskblk", [n_mask * P, QCH], BF16,
                            kind="ExternalInput").ap()
    eye_d = nc.dram_tensor("eye", [HPC, HPC * P], BF16,
                           kind="ExternalInput").ap()
    out_d = nc.dram_tensor("out", [ODPC, TOK], F32, kind="ExternalOutput").ap()

    with tile.TileContext(nc) as tc:
        with tc.tile_pool(name="persist", bufs=1) as persist, \
             tc.tile_pool(name="dram", bufs=1, space="DRAM") as dram:
            qkvT = persist.tile([P, MT, TOK], BF16)
            v_kd = persist.tile([P, NB * NKT, HD], BF16)
            ident = persist.tile([P, P], BF16)
            nc.gpsimd.memset(ident[:], 0.0)
            nc.gpsimd.affine_select(
                out=ident[:], in_=ident[:],
                compare_op=mybir.AluOpType.not_equal, fill=1.0, base=0,
                pattern=[[-1, P]], channel_multiplier=1)
            # indicator columns/rows for per-head denominator batching
            ecol = persist.tile([P, HPC, HPC], BF16)   # [:, h, :] = e_h cols
            erow = persist.tile([HPC, HPC, P], BF16)   # [:, h, :] = e_h rows
            nc.vector.memset(ecol[:], 0.0)
            for h in range(HPC):
                nc.vector.memset(ecol[:, h, h:h + 1], 1.0)
            nc.sync.dma_start(erow[:], eye_d[:])

            bounce = [dram.tile([QF, QCH], BF16, name=f"bnc{ci}")
                      for ci in range(NCHK)]
            agc = [dram.tile([QF * NCORES, QCH], BF16, name=f"agc{ci}",
                             addr_space="Shared" if NCORES > 4 else "Local")
                   for ci in range(NCHK)]

            # ---------------- Phase A: qkvT = wqkv^T @ x^T, + RoPE ----------
            with tc.tile_pool(name="pha", bufs=1) as pha, \
                 tc.tile_pool(name="phax", bufs=2) as phax, \
                 tc.tile_pool(name="phat", bufs=2) as phat, \
                 tc.tile_pool(name="psa", bufs=1, space="PSUM") as psa:
                KH = KD // 2  # stream x^T in two half-contraction chunks
                wq_sb = pha.tile([P, KD, FLOC], BF16)

                def load_xt(col0, half):
                    xt_sb = phax.tile([P, KH, QCH], BF16, tag="xt",
                                      name=f"xt{col0}_{half}")
                    nc.sync.dma_start(
                        xt_sb[:],
                        xt_d[half * KH * P:(half + 1) * KH * P,
                             col0:col0 + QCH]
                        .rearrange("(ko p) t -> p ko t", p=P))
                    return xt_sb

                # first weight slices + first x^T chunk up front, then the rest
                for ko in range(2):
                    nc.sync.dma_start(
                        wq_sb[:, ko, :], wqkv_d[ko * P:(ko + 1) * P, :])
                xt_first = load_xt(0, 0)
                for ko in range(2, KD):
                    nc.sync.dma_start(
                        wq_sb[:, ko, :], wqkv_d[ko * P:(ko + 1) * P, :])
                sc_sb = pha.tile([P, 2 * S], BF16)
                nc.sync.dma_start(sc_sb[:], sc_d[:])
                cosT = sc_sb[:, 0:S]
                sinT = sc_sb[:, S:2 * S]

                for b in range(NB):
                    for chb in range(S // QCH):
                        ch = b * (S // QCH) + chb
                        col0 = ch * QCH
                        pss = [psa.tile([P, QCH], F32, tag="pa", bufs=MT + 1,
                                         name=f"pa{ch}_{m}")
                               for m in range(MT)]
                        for half in range(2):
                            if ch == 0 and half == 0:
                                xt_sb = xt_first
                            else:
                                xt_sb = load_xt(col0, half)
                            for k in range(KH):
                                kg = half * KH + k
                                for m in range(MT):
                                    nc.tensor.matmul(
                                        pss[m][:],
                                        wq_sb[:, kg, m * P:(m + 1) * P],
                                        xt_sb[:, k, :],
                                        start=(kg == 0), stop=(kg == KD - 1))
                        # RoPE for q tiles and the k tile; plain copy for v
                        s0 = col0 % S
                        for m in range(MT):
                            dst = qkvT[:, m, col0:col0 + QCH]
                            if m == MT - 1:  # v
                                nc.vector.tensor_copy(dst, pss[m][:])
                                continue
                            t1 = phat.tile([P, QCH], F32, tag="t1")
                            t2 = phat.tile([P, QCH], F32, tag="t2")
                            nc.vector.tensor_mul(t1[:], pss[m][:],
                                                 cosT[:, s0:s0 + QCH])
                            nc.vector.tensor_mul(t2[0:HHD, :],
                                                 pss[m][HHD:P, :],
                                                 sinT[0:HHD, s0:s0 + QCH])
                            nc.vector.tensor_mul(t2[HHD:P, :],
                                                 pss[m][0:HHD, :],
                                                 sinT[HHD:P, s0:s0 + QCH])
                            nc.vector.tensor_add(dst, t1[:], t2[:])
                    # V^T -> V via PE transposes (bf16), per batch
                    for kt in range(NKT):
                        pt_ps = psa.tile([P, P], BF16, tag="vt", bufs=1,
                                          name=f"vt{b}_{kt}")
                        nc.tensor.transpose(
                            pt_ps[:],
                            qkvT[:, MT - 1,
                                 b * S + kt * KT:b * S + (kt + 1) * KT],
                            ident[:])
                        nc.vector.tensor_copy(v_kd[:, b * NKT + kt, :],
                                              pt_ps[:])

            # ------------- Phases B + AG + C, software-pipelined ------------
            with tc.tile_pool(name="phb", bufs=1) as phb, \
                 tc.tile_pool(name="phbw", bufs=3) as phbw, \
                 tc.tile_pool(name="phcx", bufs=2) as phcx, \
                 tc.tile_pool(name="phco", bufs=2) as phco, \
                 tc.tile_pool(name="psbc", bufs=1, space="PSUM") as psbc:
                wo_sb = phb.tile([P, KD, ODPC], BF16)
                for ko in range(KD):
                    nc.sync.dma_start(
                        wo_sb[:, ko, :], wo_d[ko * P:(ko + 1) * P, :])
                if n_mixed > 0:
                    mk_sb = phb.tile([P, n_mask, QCH], BF16)
                    nc.sync.dma_start(
                        mk_sb[:], mask_d.rearrange("(mb p) q -> p mb q", p=P))

                def phase_c_chunk(ci):
                    agt = phcx.tile([P, KD, QCH], BF16, tag="agt",
                                    name=f"agt{ci}")
                    nc.sync.dma_start(
                        agt[:],
                        agc[ci].rearrange("(ko p) t -> p ko t", p=P))
                    for md in range(ODPC // P):
                        po = psbc.tile([P, QCH], F32, tag="po", bufs=2,
                                       name=f"po{ci}_{md}")
                        for kf in range(KD):
                            nc.tensor.matmul(
                                po[:],
                                wo_sb[:, kf, md * P:(md + 1) * P],
                                agt[:, kf, :],
                                start=(kf == 0), stop=(kf == KD - 1))
                        osb = phco.tile([P, QCH], F32, tag="osb")
                        nc.vector.tensor_copy(osb[:], po[:])
                        nc.sync.dma_start(
                            out_d[md * P:(md + 1) * P,
                                  ci * QCH:(ci + 1) * QCH], osb[:])

                # software pipeline: B(ci) -> AG(ci) while C(ci-1) runs
                _o_tiles = {}

                def phase_b_chunk2(ci):
                    b, qc = divmod(ci, NQC)
                    kts = [kt for kt in range(NKT)
                           if block_cls[(qc, kt)] != 'skip']
                    q0 = b * S + qc * QCH
                    d_ps = psbc.tile([HPC, QCH], F32, tag="den", bufs=2,
                                     name=f"den{ci}")
                    for h in range(HPC):
                        o_ps = psbc.tile([P, QCH], F32, tag="outT", bufs=2,
                                         name=f"o{ci}_{h}")
                        for i, kt in enumerate(kts):
                            st = psbc.tile([P, QCH], F32, tag="st", bufs=2,
                                           name=f"st{ci}_{h}_{i}")
                            nc.tensor.matmul(
                                st[:],
                                qkvT[:, HPC,
                                     b * S + kt * KT:b * S + (kt + 1) * KT],
                                qkvT[:, h, q0:q0 + QCH],
                                start=True, stop=True)
                            pt = phbw.tile([P, QCH], BF16, tag="pt", bufs=6,
                                           name=f"pt{ci}_{h}_{i}")
                            nc.scalar.activation(
                                pt[:], st[:],
                                mybir.ActivationFunctionType.Exp,
                                bias=0.0, scale=float(SCALE))
                            cls = block_cls[(qc, kt)]
                            if cls != 'full':
                                nc.vector.tensor_mul(pt[:], pt[:],
                                                     mk_sb[:, cls, :])
                            first, last = (i == 0), (i == len(kts) - 1)
                            nc.tensor.matmul(
                                o_ps[:], v_kd[:, b * NKT + kt, :], pt[:],
                                start=first, stop=last)
                            # group up to 4 exp blocks per denominator matmul
                            gpos = i % 4
                            if gpos == 0:
                                dacc, dacc_n = pt, 1
                            else:
                                if dacc_n == 1:
                                    dsum = phbw.tile([P, QCH], BF16,
                                                     tag="dsum", bufs=2,
                                                     name=f"ds{ci}_{h}_{i}")
                                    nc.vector.tensor_add(dsum[:], dacc[:],
                                                         pt[:])
                                    dacc = dsum
                                else:
                                    nc.vector.tensor_add(dacc[:], dacc[:],
                                                         pt[:])
                                dacc_n += 1
                            if gpos == 3 or last:
                                nc.tensor.matmul(
                                    d_ps[:], ecol[:, h, :], dacc[:],
                                    start=(i < 4 and h == 0),
                                    stop=(last and h == HPC - 1))
                        o_sb = phbw.tile([P, QCH], BF16, tag="osbuf", bufs=5,
                                         name=f"ou{ci}_{h}")
                        nc.vector.tensor_copy(o_sb[:], o_ps[:])
                        _o_tiles[(ci, h)] = o_sb
                    inv = phbw.tile([HPC, QCH], F32, tag="inv",
                                    name=f"inv{ci}")
                    nc.vector.reciprocal(inv[:], d_ps[:])
                    invb = phbw.tile([HPC, QCH], BF16, tag="invb",
                                     name=f"invb{ci}")
                    nc.vector.tensor_copy(invb[:], inv[:])
                    # normalize + stage each head's block to the bounce buffer
                    for h in range(HPC):
                        bc_ps = psbc.tile([P, QCH], F32, tag="st", bufs=2,
                                          name=f"bc{ci}_{h}")
                        nc.tensor.matmul(bc_ps[:], erow[:, h, :], invb[:],
                                         start=True, stop=True)
                        bcc = phbw.tile([P, QCH], BF16, tag="bcc",
                                        name=f"bcc{ci}_{h}")
                        nc.vector.tensor_copy(bcc[:], bc_ps[:])
                        at = phbw.tile([P, QCH], BF16, tag="at",
                                       name=f"at{ci}_{h}")
                        nc.vector.tensor_mul(at[:], _o_tiles[(ci, h)][:],
                                             bcc[:])
                        nc.sync.dma_start(
                            bounce[ci][h * P:(h + 1) * P, :], at[:])

                for ci in range(NCHK):
                    phase_b_chunk2(ci)
                    nc.gpsimd.collective_compute(
                        "AllGather", mybir.AluOpType.bypass,
                        replica_groups=[list(range(NCORES))],
                        ins=[bounce[ci].opt()], outs=[agc[ci].opt()])
                    if ci >= 1:
                        phase_c_chunk(ci - 1)
                phase_c_chunk(NCHK - 1)

    nc.compile()
    return nc


def _host_prep(x, wqkv, wo, sincos, full_causal_mask, start_pos,
               NB, S, D, HPC, NCORES):
    """Shard, cast, and lay out inputs; classify mask blocks."""
    bf16 = ml_dtypes.bfloat16
    TOK = NB * S
    H = HPC * NCORES
    QF = HPC * HD
    NQC = S // QCH
    NKT = S // KT
    ODPC = D // NCORES
    q_sz = H * HD

    xt = np.ascontiguousarray(x.reshape(TOK, D).T).astype(bf16)

    # effective mask: [q, k] (batch-shared), incl. the cache-validity term
    m_eff = np.asarray(full_causal_mask[0, 0], dtype=bool)
    m_eff = m_eff[start_pos:start_pos + S, :S].copy()
    valid = np.arange(S) < (start_pos + S)
    m_eff &= valid[None, :]

    block_cls = {}
    mixed_blocks = []
    for qc in range(NQC):
        for kt in range(NKT):
            blk = m_eff[qc * QCH:(qc + 1) * QCH, kt * KT:(kt + 1) * KT]
            if blk.all():
                block_cls[(qc, kt)] = 'full'
            elif not blk.any():
                block_cls[(qc, kt)] = 'skip'
            else:
                block_cls[(qc, kt)] = len(mixed_blocks)
                mixed_blocks.append(
                    np.ascontiguousarray(blk.T.astype(np.float32)))  # [k, q]
    n_mixed = len(mixed_blocks)
    if n_mixed:
        maskblk = np.concatenate(mixed_blocks, axis=0).astype(bf16)
    else:
        maskblk = np.zeros((P, QCH), dtype=bf16)

    # rope tables, transposed + duplicated halves; sin rows 0:64 negated
    sc = np.asarray(sincos[start_pos:start_pos + S], dtype=np.float32)
    sin, cos = sc[:, :HHD], sc[:, HHD:]
    cosT2 = np.concatenate([cos.T, cos.T], axis=0)           # [128, S]
    sinT2 = np.concatenate([-sin.T, sin.T], axis=0)          # [128, S]
    sincos2 = np.concatenate([cosT2, sinT2], axis=1).astype(bf16)

    eye = np.zeros((HPC, HPC, P), dtype=bf16)
    for h in range(HPC):
        eye[h, h, :] = 1
    eye = eye.reshape(HPC, HPC * P)

    in_maps = []
    for c in range(NCORES):
        qcols = np.asarray(wqkv[:, c * QF:(c + 1) * QF])
        kcols = np.asarray(wqkv[:, q_sz + c * HD:q_sz + (c + 1) * HD])
        vcols = np.asarray(
            wqkv[:, q_sz + NCORES * HD + c * HD:
                 q_sz + NCORES * HD + (c + 1) * HD])
        wqkv_c = np.concatenate([qcols, kcols, vcols], axis=1).astype(bf16)
        wo_c = np.ascontiguousarray(
            np.asarray(wo[:, c * ODPC:(c + 1) * ODPC])).astype(bf16)
        in_maps.append({
            "xt": xt, "wqkv": wqkv_c, "wo": wo_c,
            "sincos2": sincos2, "maskblk": maskblk, "eye": eye,
        })
    return in_maps, block_cls, n_mixed


_CACHE = {}


def run_distributed(x, wqkv, wo, sincos, full_causal_mask, start_pos,
                    NB, S, D, HPC, NCORES, trace=False, tmpdir=None):
    in_maps, block_cls, n_mixed = _host_prep(
        x, wqkv, wo, sincos, full_causal_mask, start_pos,
        NB, S, D, HPC, NCORES)
    key = (NB, S, D, HPC, NCORES,
           tuple(sorted((k, v) for k, v in block_cls.items())))
    if key not in _CACHE:
        _CACHE[key] = build_graph(NB, S, D, HPC, NCORES, block_cls, n_mixed)
    nc = _CACHE[key]
    res = run_bass_kernel_spmd(nc, in_maps, list(range(NCORES)), trace=trace,
                               tmpdir=tmpdir)
    TOK = NB * S
    out = np.empty((TOK, D), dtype=np.float32)
    ODPC = D // NCORES
    for c in range(NCORES):
        out[:, c * ODPC:(c + 1) * ODPC] = res.results[c]["out"].T
    return out.reshape(NB, S, D), res


def kernel(x, wqkv, wo, sincos, cache_k, cache_v, full_causal_mask,
           start_pos) -> np.ndarray:
    x = np.asarray(x)
    start_pos = int(np.asarray(start_pos))
    B, S_, D_ = x.shape
    assert start_pos == 0, "prefill-only kernel (seq fills the whole cache)"
    out, _ = run_distributed(
        x, np.asarray(wqkv), np.asarray(wo), np.asarray(sincos),
        np.asarray(full_causal_mask), start_pos,
        NB=B, S=S_, D=D_, HPC=4, NCORES=8)
    return out
